# revision 10
# baseline (speedup 1.0000x reference)
"""MoE router (AutonomousRouter) for TRN2, 8 NeuronCores.

Computes reference:
    act    = einsum('bnd,edc->bnec', x, W)          B,N,D,E,C = 4,2048,2048,8,512
    logits = ||act||_2 over c                       [B,N,E]
    probs  = softmax(logits, -1)
    top-2 routing with capacity 640 (priority = order within k-major (choice, token) sequence)
    out    = stack([dispatch, combine])             [2,B,N,E,640] fp32

Sharding: data-parallel over tokens; core i <- tokens [i*1024, (i+1)*1024) of the
flattened [8192] token axis (= batch b=i//2, half i%2). Weights replicated.

Phase A (device): single-pass bf16 matmuls -> sum-of-squares per (token, expert).
Host glue: routing control. bf16 rounding perturbs sumsq by <~0.45 abs (scale ~420);
tokens whose top-3 sumsq gaps fall inside the MARGIN (~10-15%) are recomputed in
fp64 on host so the top-2 selection/order matches the fp32 reference exactly
(with the same sub-ulp lower-index tie-break the reference's fp32 noise selects).
Softmax, top-2, and the k-major capacity cumsum are tiny [8192,8] numpy ops.
Phase B (device): dispatch/combine have <=2 nonzero scalars per (token, expert)
row; scatter exactly those elements (value 1.0 resp. prob) into the pre-zeroed
dense outputs via per-element indirect DMA. Capacity-overflow entries are
redirected to a spill tail that the host slices off.
"""
import numpy as np

import concourse.bacc as bacc
import concourse.mybir as mybir
from concourse.tile import TileContext
from concourse.bass_utils import run_bass_kernel_spmd

P = 128          # partitions
B, N, D, E, C = 4, 2048, 2048, 8, 512
CAP = 640
NCORES = 8
TOK = (B * N) // NCORES     # tokens per core = 1024
NT = TOK // P               # token tiles per core = 8
KT = D // P                 # contraction tiles = 16
MARGIN = 1.25               # sumsq gap below which host recomputes in fp64

f32 = mybir.dt.float32

_cache = {}
LAST_IN_MAPS_A = None   # kept for test harness re-runs/profiling
LAST_IN_MAPS_B = None


def _build_phase_a():
    bf16 = mybir.dt.bfloat16
    nc = bacc.Bacc("TRN2", target_bir_lowering=False, debug=False, num_devices=NCORES)
    xT = nc.dram_tensor("xT", [D, TOK], bf16, kind="ExternalInput")
    w = nc.dram_tensor("w", [E, D, C], bf16, kind="ExternalInput")
    ss_out = nc.dram_tensor("ss", [TOK, E], f32, kind="ExternalOutput")

    with TileContext(nc) as tc:
        with (
            tc.tile_pool(name="const", bufs=1) as cpool,
            tc.tile_pool(name="wbuf", bufs=2) as wpool,
            tc.tile_pool(name="work", bufs=3) as spool,
            tc.tile_pool(name="psum", bufs=8, space="PSUM") as psum,
        ):
            # x^T resident in variable k-chunk tiles; W per expert likewise
            # (double-buffered). DMAs are issued in consumption order and the
            # first chunk is a single k-block, so the first matmuls wait on
            # ~0.4MB instead of the full 20MB.
            CHUNKS = [1, 3, 4, 4, 4]           # k-blocks per chunk, sums to KT
            CH0 = [sum(CHUNKS[:i]) for i in range(len(CHUNKS))]  # chunk k-starts
            NCH = len(CHUNKS)

            def _x_chunk(q):
                nk = CHUNKS[q]
                name = f"xq{q}"
                tile_ = cpool.tile([P, nk * TOK], bf16, tag=name, name=name)
                nc.sync.dma_start(
                    out=tile_[:].rearrange("p (k n) -> p k n", k=nk),
                    in_=xT.ap()[CH0[q] * P:(CH0[q] + nk) * P, :]
                        .rearrange("(k p) n -> p k n", p=P),
                )
                return tile_

            def _w_chunk(e, q):
                nk = CHUNKS[q]
                tile_ = wpool.tile([P, nk * C], bf16, tag=f"wq{q}", name=f"w{e}_{q}")
                nc.sync.dma_start(
                    out=tile_[:].rearrange("p (k c) -> p k c", k=nk),
                    in_=w.ap()[e, CH0[q] * P:(CH0[q] + nk) * P, :]
                        .rearrange("(k p) c -> p k c", p=P),
                )
                return tile_

            # consumption-order issue: W(e0,q0), x(q0), W(e0,q1), x(q1), ...
            w0_q, x_q = [], []
            for q in range(NCH):
                w0_q.append(_w_chunk(0, q))
                x_q.append(_x_chunk(q))

            # per-token-tile sum-of-squares accumulators [128, E]
            ss_tiles = [cpool.tile([P, E], f32, tag=f"ss{t}", name=f"ss{t}")
                        for t in range(NT)]

            # ---- matmul phase: for each expert, 8 token tiles x 16 k-tiles ----
            for e in range(E):
                w_q = w0_q if e == 0 else [_w_chunk(e, q) for q in range(NCH)]
                for t in range(NT):
                    ps = psum.tile([P, C], f32, space="PSUM", tag="ps")
                    for k in range(KT):
                        q = max(i for i in range(NCH) if CH0[i] <= k)
                        kq = k - CH0[q]
                        x_blk = x_q[q][:, kq * TOK + t * P: kq * TOK + (t + 1) * P]
                        w_blk = w_q[q][:, kq * C:(kq + 1) * C]
                        nc.tensor.matmul(ps[:], lhsT=x_blk, rhs=w_blk,
                                         start=(k == 0), stop=(k == KT - 1))
                    sq = spool.tile([P, C], f32, tag="sq")
                    nc.scalar.activation(sq[:], ps[:], mybir.ActivationFunctionType.Square)
                    red8 = spool.tile([P, 8], f32, tag="red8")
                    nc.vector.tensor_reduce(
                        red8[:], sq[:].rearrange("p (g c) -> p g c", g=8),
                        axis=mybir.AxisListType.X, op=mybir.AluOpType.add,
                    )
                    nc.vector.tensor_reduce(
                        ss_tiles[t][:, e:e + 1], red8[:],
                        axis=mybir.AxisListType.X, op=mybir.AluOpType.add,
                    )
            for t in range(NT):
                nc.sync.dma_start(out=ss_out.ap()[t * P:(t + 1) * P, :], in_=ss_tiles[t][:])
    nc.compile()
    return nc


def _build_phase_b(cap=CAP):
    """Paired element scatter: each (token, choice) contributes exactly one
    nonzero scalar to dispatch (1.0) and combine (prob), both at flat index
    (t*E + e)*cap + slot of their plane. Interleave the planes as [rows, 2]
    so a single 8-byte descriptor writes both; host de-interleaves. 2048
    descriptors total, ~16KB written, no one-hot row materialization."""
    import concourse.bass as bass
    i32 = mybir.dt.int32
    NR = 2 * TOK          # (token x choice) pairs per core
    NG = NR // P          # 16 scatter groups of 128 pairs
    SPILL = NR            # capacity-overflow pairs land past the real output
    nc = bacc.Bacc("TRN2", target_bir_lowering=False, debug=False, num_devices=NCORES)
    fidx = nc.dram_tensor("fidx", [NR, 1], i32, kind="ExternalInput")
    vals = nc.dram_tensor("vals", [NR, 2], f32, kind="ExternalInput")
    # two disjoint-row output tensors (merged by host add): alternating targets
    # keeps Tile's same-tensor WAW chains interleaved, hiding each scatter's
    # SDMA completion wait under the other chain's descriptor generation
    dcA = nc.dram_tensor("dcA", [TOK * E * cap + SPILL, 2], f32, kind="ExternalOutput")
    dcB = nc.dram_tensor("dcB", [TOK * E * cap + SPILL, 2], f32, kind="ExternalOutput")

    with TileContext(nc) as tc:
        with tc.tile_pool(name="const", bufs=1) as cpool:
            fi = cpool.tile([P, NG], i32, tag="fi")
            nc.sync.dma_start(out=fi[:], in_=fidx.ap()[:, 0].rearrange("(g p) -> p g", p=P))
            vv = cpool.tile([P, NG * 2], f32, tag="vv")
            nc.sync.dma_start(
                out=vv[:].rearrange("p (g v) -> p g v", g=NG),
                in_=vals.ap()[:, :].rearrange("(g p) v -> p g v", p=P))
            for g in range(NG):
                dc = dcA if g % 2 == 0 else dcB
                nc.gpsimd.indirect_dma_start(
                    out=dc.ap()[:, :],
                    out_offset=bass.IndirectOffsetOnAxis(ap=fi[:, g:g + 1], axis=0),
                    in_=vv[:, 2 * g:2 * g + 2], in_offset=None)
    nc.compile()
    return nc


def _get(name, builder):
    if name not in _cache:
        _cache[name] = builder()
    return _cache[name]


def kernel(token_inputs, bottleneck_weights, expert_capacity):
    import ml_dtypes
    x = np.ascontiguousarray(np.asarray(token_inputs, dtype=np.float32)).reshape(B * N, D)
    w = np.ascontiguousarray(np.asarray(bottleneck_weights, dtype=np.float32))
    cap = int(expert_capacity)
    assert cap > 0

    wb = w.astype(ml_dtypes.bfloat16)
    core_ids = list(range(NCORES))
    in_maps_a = []
    for c in core_ids:
        shard_t = np.ascontiguousarray(x[c * TOK:(c + 1) * TOK].T)   # [2048, 1024]
        in_maps_a.append({"xT": shard_t.astype(ml_dtypes.bfloat16), "w": wb})

    global LAST_IN_MAPS_A, LAST_IN_MAPS_B
    LAST_IN_MAPS_A = in_maps_a
    nc_a = _get("a", _build_phase_a)
    res_a = run_bass_kernel_spmd(nc_a, in_maps_a, core_ids)

    # ---- host glue: routing control on [8192, 8] scalars ----
    ss = np.concatenate([res_a.results[c]["ss"] for c in core_ids], axis=0)  # [8192, 8]

    # fp64 recompute of tokens whose bf16-grade sumsq cannot certify the
    # reference's fp32 top-2 selection/order (top-3 gaps inside MARGIN)
    ss = ss.astype(np.float64)
    srt = np.sort(ss, axis=1)
    flag = ((srt[:, -1] - srt[:, -2]) < MARGIN) | ((srt[:, -2] - srt[:, -3]) < MARGIN)
    if flag.any():
        xf = x[flag].astype(np.float64)
        for e in range(E):
            act = xf @ w[e].astype(np.float64)
            ss[flag, e] = np.einsum('tc,tc->t', act, act)

    logits = np.sqrt(ss)
    ex = np.exp(logits - logits.max(axis=1, keepdims=True))
    probs = ex / ex.sum(axis=1, keepdims=True)            # float64, monotone in ss
    # selection on ss - e*1e-4: sub-ulp lower-index bias so fp32-noise-level
    # near-ties resolve the way the reference's fp32 top_k resolves them
    order = np.argsort(-(ss - 1e-4 * np.arange(E)), axis=1, kind='stable')
    e0g, e1g = order[:, 0], order[:, 1]

    ar = np.arange(TOK)
    base = TOK * E * cap
    in_maps_b = []
    for b in range(B):
        sl = slice(b * N, (b + 1) * N)
        e0b, e1b = e0g[sl], e1g[sl]
        seq = np.concatenate([e0b, e1b])                  # k-major (choice, token)
        slots = np.empty(2 * N, np.int64)
        for e in range(E):
            m = seq == e
            slots[m] = np.arange(m.sum())
        pb = probs[sl]
        for h in range(2):
            tg = h * TOK + ar                             # batch-local token idx
            s0, s1 = slots[:N][tg], slots[N:][tg]
            ee0, ee1 = e0b[tg], e1b[tg]
            fi0 = np.where(s0 < cap, (ar * E + ee0) * cap + s0, base + ar)
            fi1 = np.where(s1 < cap, (ar * E + ee1) * cap + s1, base + TOK + ar)
            pv = np.concatenate([pb[tg, ee0], pb[tg, ee1]]).astype(np.float32)
            vals = np.empty((2 * TOK, 2), np.float32)
            vals[:, 0] = 1.0
            vals[:, 1] = pv
            in_maps_b.append({
                "fidx": np.concatenate([fi0, fi1]).astype(np.int32)[:, None],
                "vals": vals,
            })

    LAST_IN_MAPS_B = in_maps_b
    nc_b = _get(f"b{cap}", lambda: _build_phase_b(cap))
    res_b = run_bass_kernel_spmd(nc_b, in_maps_b, core_ids)

    out = np.empty((2, B, N, E, cap), np.float32)
    for c in core_ids:
        b, h = c // 2, c % 2
        sl = slice(h * TOK, (h + 1) * TOK)
        dcp = res_b.results[c]["dcA"][:base] + res_b.results[c]["dcB"][:base]
        out[0, b, sl] = dcp[:, 0].reshape(TOK, E, cap)         # [base, 2] interleaved
        out[1, b, sl] = dcp[:, 1].reshape(TOK, E, cap)
    return out


# revision 13
# speedup vs baseline: 1.1913x; 1.1913x over previous
"""MoE router (AutonomousRouter) for TRN2, 8 NeuronCores.

Computes reference:
    act    = einsum('bnd,edc->bnec', x, W)          B,N,D,E,C = 4,2048,2048,8,512
    logits = ||act||_2 over c                       [B,N,E]
    probs  = softmax(logits, -1)
    top-2 routing with capacity 640 (priority = order within k-major (choice, token) sequence)
    out    = stack([dispatch, combine])             [2,B,N,E,640] fp32

Sharding: data-parallel over tokens; core i <- tokens [i*1024, (i+1)*1024) of the
flattened [8192] token axis (= batch b=i//2, half i%2). Weights replicated.

Phase A (device): single-pass bf16 matmuls -> sum-of-squares per (token, expert).
Host glue: routing control. bf16 rounding perturbs sumsq by <~0.45 abs (scale ~420);
tokens whose top-3 sumsq gaps fall inside the MARGIN (~10-15%) are recomputed in
fp64 on host so the top-2 selection/order matches the fp32 reference exactly
(with the same sub-ulp lower-index tie-break the reference's fp32 noise selects).
Softmax, top-2, and the k-major capacity cumsum are tiny [8192,8] numpy ops.
Phase B (device): dispatch/combine have <=2 nonzero scalars per (token, expert)
row; scatter exactly those elements (value 1.0 resp. prob) into the pre-zeroed
dense outputs via per-element indirect DMA. Capacity-overflow entries are
redirected to a spill tail that the host slices off.
"""
import numpy as np

import concourse.bacc as bacc
import concourse.mybir as mybir
from concourse.tile import TileContext
from concourse.bass_utils import run_bass_kernel_spmd

P = 128          # partitions
B, N, D, E, C = 4, 2048, 2048, 8, 512
CAP = 640
NCORES = 8
TOK = (B * N) // NCORES     # tokens per core = 1024
NT = TOK // P               # token tiles per core = 8
KT = D // P                 # contraction tiles = 16
MARGIN = 1.25               # sumsq gap below which host recomputes in fp64

f32 = mybir.dt.float32

_cache = {}
LAST_IN_MAPS_A = None   # kept for test harness re-runs/profiling
LAST_IN_MAPS_B = None


def _build_phase_a():
    bf16 = mybir.dt.bfloat16
    nc = bacc.Bacc("TRN2", target_bir_lowering=False, debug=False, num_devices=NCORES)
    xT = nc.dram_tensor("xT", [D, TOK], bf16, kind="ExternalInput")
    w = nc.dram_tensor("w", [E, D, C], bf16, kind="ExternalInput")
    ss_out = nc.dram_tensor("ss", [TOK, E], f32, kind="ExternalOutput")

    with TileContext(nc) as tc:
        with (
            tc.tile_pool(name="const", bufs=1) as cpool,
            tc.tile_pool(name="wbuf", bufs=2) as wpool,
            tc.tile_pool(name="work", bufs=3) as spool,
            tc.tile_pool(name="psum", bufs=8, space="PSUM") as psum,
        ):
            # x^T resident in variable k-chunk tiles; W per expert likewise
            # (double-buffered). DMAs are issued in consumption order and the
            # first chunk is a single k-block, so the first matmuls wait on
            # ~0.4MB instead of the full 20MB.
            CHUNKS = [1, 3, 4, 4, 4]           # k-blocks per chunk, sums to KT
            CH0 = [sum(CHUNKS[:i]) for i in range(len(CHUNKS))]  # chunk k-starts
            NCH = len(CHUNKS)

            def _x_chunk(q):
                nk = CHUNKS[q]
                name = f"xq{q}"
                tile_ = cpool.tile([P, nk * TOK], bf16, tag=name, name=name)
                nc.sync.dma_start(
                    out=tile_[:].rearrange("p (k n) -> p k n", k=nk),
                    in_=xT.ap()[CH0[q] * P:(CH0[q] + nk) * P, :]
                        .rearrange("(k p) n -> p k n", p=P),
                )
                return tile_

            def _w_chunk(e, q):
                nk = CHUNKS[q]
                tile_ = wpool.tile([P, nk * C], bf16, tag=f"wq{q}", name=f"w{e}_{q}")
                nc.sync.dma_start(
                    out=tile_[:].rearrange("p (k c) -> p k c", k=nk),
                    in_=w.ap()[e, CH0[q] * P:(CH0[q] + nk) * P, :]
                        .rearrange("(k p) c -> p k c", p=P),
                )
                return tile_

            # consumption-order issue: W(e0,q0), x(q0), W(e0,q1), x(q1), ...
            w0_q, x_q = [], []
            for q in range(NCH):
                w0_q.append(_w_chunk(0, q))
                x_q.append(_x_chunk(q))

            # HAM pre-warm: ~3.5us of dummy matmuls on a memset tile while the
            # first input chunks are still in flight, so the free-running
            # activity window un-throttles the PE clock (1.2 -> 2.4 GHz)
            # before the real stream starts.
            warm = cpool.tile([P, P], bf16, tag="warm")
            nc.vector.memset(warm[:], 0.0)
            wps = psum.tile([P, C], f32, space="PSUM", tag="ps")
            for _ in range(36):
                nc.tensor.matmul(wps[:, 0:P], lhsT=warm[:], rhs=warm[:],
                                 start=True, stop=True)

            # per-token-tile sum-of-squares accumulators [128, E]
            ss_tiles = [cpool.tile([P, E], f32, tag=f"ss{t}", name=f"ss{t}")
                        for t in range(NT)]

            # ---- matmul phase: for each expert, 8 token tiles x 16 k-tiles ----
            for e in range(E):
                w_q = w0_q if e == 0 else [_w_chunk(e, q) for q in range(NCH)]
                for t in range(NT):
                    ps = psum.tile([P, C], f32, space="PSUM", tag="ps")
                    for k in range(KT):
                        q = max(i for i in range(NCH) if CH0[i] <= k)
                        kq = k - CH0[q]
                        x_blk = x_q[q][:, kq * TOK + t * P: kq * TOK + (t + 1) * P]
                        w_blk = w_q[q][:, kq * C:(kq + 1) * C]
                        nc.tensor.matmul(ps[:], lhsT=x_blk, rhs=w_blk,
                                         start=(k == 0), stop=(k == KT - 1))
                    sq = spool.tile([P, C], f32, tag="sq")
                    nc.scalar.activation(sq[:], ps[:], mybir.ActivationFunctionType.Square)
                    red8 = spool.tile([P, 8], f32, tag="red8")
                    nc.vector.tensor_reduce(
                        red8[:], sq[:].rearrange("p (g c) -> p g c", g=8),
                        axis=mybir.AxisListType.X, op=mybir.AluOpType.add,
                    )
                    nc.vector.tensor_reduce(
                        ss_tiles[t][:, e:e + 1], red8[:],
                        axis=mybir.AxisListType.X, op=mybir.AluOpType.add,
                    )
                    if e == E - 1:
                        # ss[t] complete — stream it out under the remaining tiles
                        nc.sync.dma_start(out=ss_out.ap()[t * P:(t + 1) * P, :],
                                          in_=ss_tiles[t][:])
    nc.compile()
    return nc


def _build_phase_b(cap=CAP):
    """Paired element scatter: each (token, choice) contributes exactly one
    nonzero scalar to dispatch (1.0) and combine (prob), both at flat index
    (t*E + e)*cap + slot of their plane. Interleave the planes as [rows, 2]
    so a single 8-byte descriptor writes both; host de-interleaves. 2048
    descriptors total, ~16KB written, no one-hot row materialization."""
    import concourse.bass as bass
    i32 = mybir.dt.int32
    NR = 2 * TOK          # (token x choice) pairs per core
    NG = NR // P          # 16 scatter groups of 128 pairs
    SPILL = NR            # capacity-overflow pairs land past the real output
    nc = bacc.Bacc("TRN2", target_bir_lowering=False, debug=False, num_devices=NCORES)
    fidx = nc.dram_tensor("fidx", [NR, 1], i32, kind="ExternalInput")
    vals = nc.dram_tensor("vals", [NR, 2], f32, kind="ExternalInput")
    # two disjoint-row output tensors (merged by host add): alternating targets
    # keeps Tile's same-tensor WAW chains interleaved, hiding each scatter's
    # SDMA completion wait under the other chain's descriptor generation
    dcA = nc.dram_tensor("dcA", [TOK * E * cap + SPILL, 2], f32, kind="ExternalOutput")
    dcB = nc.dram_tensor("dcB", [TOK * E * cap + SPILL, 2], f32, kind="ExternalOutput")

    with TileContext(nc) as tc:
        with tc.tile_pool(name="const", bufs=1) as cpool:
            fi = cpool.tile([P, NG], i32, tag="fi")
            nc.sync.dma_start(out=fi[:], in_=fidx.ap()[:, 0].rearrange("(g p) -> p g", p=P))
            vv = cpool.tile([P, NG * 2], f32, tag="vv")
            nc.sync.dma_start(
                out=vv[:].rearrange("p (g v) -> p g v", g=NG),
                in_=vals.ap()[:, :].rearrange("(g p) v -> p g v", p=P))
            for g in range(NG):
                dc = dcA if g % 2 == 0 else dcB
                nc.gpsimd.indirect_dma_start(
                    out=dc.ap()[:, :],
                    out_offset=bass.IndirectOffsetOnAxis(ap=fi[:, g:g + 1], axis=0),
                    in_=vv[:, 2 * g:2 * g + 2], in_offset=None)
    nc.compile()
    return nc


def _get(name, builder):
    if name not in _cache:
        _cache[name] = builder()
    return _cache[name]


def kernel(token_inputs, bottleneck_weights, expert_capacity):
    import ml_dtypes
    x = np.ascontiguousarray(np.asarray(token_inputs, dtype=np.float32)).reshape(B * N, D)
    w = np.ascontiguousarray(np.asarray(bottleneck_weights, dtype=np.float32))
    cap = int(expert_capacity)
    assert cap > 0

    wb = w.astype(ml_dtypes.bfloat16)
    core_ids = list(range(NCORES))
    in_maps_a = []
    for c in core_ids:
        shard_t = np.ascontiguousarray(x[c * TOK:(c + 1) * TOK].T)   # [2048, 1024]
        in_maps_a.append({"xT": shard_t.astype(ml_dtypes.bfloat16), "w": wb})

    global LAST_IN_MAPS_A, LAST_IN_MAPS_B
    LAST_IN_MAPS_A = in_maps_a
    nc_a = _get("a", _build_phase_a)
    res_a = run_bass_kernel_spmd(nc_a, in_maps_a, core_ids)

    # ---- host glue: routing control on [8192, 8] scalars ----
    ss = np.concatenate([res_a.results[c]["ss"] for c in core_ids], axis=0)  # [8192, 8]

    # fp64 recompute of tokens whose bf16-grade sumsq cannot certify the
    # reference's fp32 top-2 selection/order (top-3 gaps inside MARGIN)
    ss = ss.astype(np.float64)
    srt = np.sort(ss, axis=1)
    flag = ((srt[:, -1] - srt[:, -2]) < MARGIN) | ((srt[:, -2] - srt[:, -3]) < MARGIN)
    if flag.any():
        xf = x[flag].astype(np.float64)
        for e in range(E):
            act = xf @ w[e].astype(np.float64)
            ss[flag, e] = np.einsum('tc,tc->t', act, act)

    logits = np.sqrt(ss)
    ex = np.exp(logits - logits.max(axis=1, keepdims=True))
    probs = ex / ex.sum(axis=1, keepdims=True)            # float64, monotone in ss
    # selection on ss - e*1e-4: sub-ulp lower-index bias so fp32-noise-level
    # near-ties resolve the way the reference's fp32 top_k resolves them
    order = np.argsort(-(ss - 1e-4 * np.arange(E)), axis=1, kind='stable')
    e0g, e1g = order[:, 0], order[:, 1]

    ar = np.arange(TOK)
    base = TOK * E * cap
    in_maps_b = []
    for b in range(B):
        sl = slice(b * N, (b + 1) * N)
        e0b, e1b = e0g[sl], e1g[sl]
        seq = np.concatenate([e0b, e1b])                  # k-major (choice, token)
        slots = np.empty(2 * N, np.int64)
        for e in range(E):
            m = seq == e
            slots[m] = np.arange(m.sum())
        pb = probs[sl]
        for h in range(2):
            tg = h * TOK + ar                             # batch-local token idx
            s0, s1 = slots[:N][tg], slots[N:][tg]
            ee0, ee1 = e0b[tg], e1b[tg]
            fi0 = np.where(s0 < cap, (ar * E + ee0) * cap + s0, base + ar)
            fi1 = np.where(s1 < cap, (ar * E + ee1) * cap + s1, base + TOK + ar)
            pv = np.concatenate([pb[tg, ee0], pb[tg, ee1]]).astype(np.float32)
            vals = np.empty((2 * TOK, 2), np.float32)
            vals[:, 0] = 1.0
            vals[:, 1] = pv
            in_maps_b.append({
                "fidx": np.concatenate([fi0, fi1]).astype(np.int32)[:, None],
                "vals": vals,
            })

    LAST_IN_MAPS_B = in_maps_b
    nc_b = _get(f"b{cap}", lambda: _build_phase_b(cap))
    res_b = run_bass_kernel_spmd(nc_b, in_maps_b, core_ids)

    out = np.empty((2, B, N, E, cap), np.float32)
    for c in core_ids:
        b, h = c // 2, c % 2
        sl = slice(h * TOK, (h + 1) * TOK)
        dcp = res_b.results[c]["dcA"][:base] + res_b.results[c]["dcB"][:base]
        out[0, b, sl] = dcp[:, 0].reshape(TOK, E, cap)         # [base, 2] interleaved
        out[1, b, sl] = dcp[:, 1].reshape(TOK, E, cap)
    return out
